# revision 23
# baseline (speedup 1.0000x reference)
"""Trainium2 Bass kernel: batched multi-head self-attention (B=16, N=1024, D=768, H=12).

Strategy
--------
Data-parallel over the batch: 16 batches / 8 NeuronCores = 2 batches per core.
Each core runs an identical (SPMD) Bass program over its shard.

Per-core math, all matmuls in bf16 with fp32 PSUM accumulation:
  * Host pre-transposes x to xT [D, T] (T = 2048 local tokens) and casts
    x / qkv_w / proj_w to bf16.  qkv_w's Q|K columns are permuted on host
    into (K0,Q0,K1,Q1,...,K5,Q5) slot order so the first DMA chunks carry
    exactly what the first attention head-pair needs.
      - Q^T,K^T [c, tok] = matmul(lhsT=wqkv[:, slot], rhs=xT)
      - V [tok, c]       = matmul(lhsT=xT[:, tok-tile], rhs=wqkv_v)
      - S^T [k, q]       = matmul(lhsT=K^T_h [hd, k-tile], rhs=Q^T_h [hd, q])
        head pairs 2i/2i+1 live at partition bases 0/64 -> row-tiles
        (0,0)/(64,0) of the 64x128 PE config, streamed as adjacent pairs.
      - exp on ScalarE straight out of PSUM, bf16 into SBUF
      - out^T [hd, q]    = matmul(lhsT=[V_h | ones(64)], rhs=expT [k, q]);
        psum rows 64-127 = softmax denominator, replicated for free.
      - normalize: copy denominator to SBUF (reciprocal_approx_fast's
        BITWISE_NOT seed needs raw fp32 bits; PSUM reads don't deliver
        those on HW), reciprocal, tensor_mul -> outT (bf16)
      - y [tok, e]       = matmul(lhsT=out^T, rhs=proj_w) + bias -> bf16 DMA

Scheduling: the Tile framework scheduler is dependency-driven with a
priority heap per engine, so ordering is controlled via priority BANDS:
the attention spine (S^T pairs, exp, A@V, normalize) gets the lowest
priority numbers and preempts whenever its dependencies are satisfied;
QKV / V / proj matmul units sit in higher bands and automatically
backfill every PE bubble (exp latency, psum-ring waits, DMA waits).
Attention starts ~7us in: the first head-pair's K/Q/V units and their
DMA chunks are emitted first.

kernel() takes full unsharded inputs, shards on host, runs all 8 cores via
run_bass_kernel_spmd, and re-assembles the full output.
"""

import numpy as np
import ml_dtypes

import concourse.bass as bass
import concourse.mybir as mybir
import concourse.tile as tile
from concourse import bacc
from concourse.bass_utils import run_bass_kernel_spmd

BF16 = mybir.dt.bfloat16
F32 = mybir.dt.float32

N_CORES = 8
B, SEQ, D = 16, 1024, 768
H, HD = 12, 64
BPC = B // N_CORES            # batches per core
T = BPC * SEQ                 # tokens per core
P = 128
KT = D // P                   # 6 contraction sub-tiles of 128
NQ = 512                      # moving free-dim per matmul (1 psum bank of fp32)
QT = SEQ // NQ                # 2 query tiles per batch
KTT = SEQ // P                # 8 key-token tiles per batch
NV = 384                      # V-projection output tile (2 per 768)
SCALE = HD ** -0.5


def _emit(tc, xT_d, wqkv_d, wproj_d, bias_d, y_d):
    nc = tc.nc
    from contextlib import ExitStack

    def band(n):
        tc.cur_priority = n

    with ExitStack() as ctx:
        consts = ctx.enter_context(tc.tile_pool(name="consts", bufs=1))
        xt_pool = ctx.enter_context(tc.tile_pool(name="xt", bufs=2))
        qk_pool = ctx.enter_context(tc.tile_pool(name="qkT", bufs=2))
        v_pool = ctx.enter_context(tc.tile_pool(name="v", bufs=2))
        ot_pool = ctx.enter_context(tc.tile_pool(name="ot", bufs=2))
        e_pool = ctx.enter_context(tc.tile_pool(name="e", bufs=5))
        dn_pool = ctx.enter_context(tc.tile_pool(name="dn", bufs=2))
        rb_pool = ctx.enter_context(tc.tile_pool(name="rb", bufs=2))
        y_pool = ctx.enter_context(tc.tile_pool(name="y", bufs=2))
        mm_ps = ctx.enter_context(tc.tile_pool(name="mmps", bufs=2, space="PSUM"))
        st_ps = ctx.enter_context(tc.tile_pool(name="stps", bufs=2, space="PSUM"))
        av_ps = ctx.enter_context(tc.tile_pool(name="avps", bufs=2, space="PSUM"))

        # ---------------- DMA (need-ordered chunks), band 0 ----------------
        band(0)
        xT_full = xT_d[:].rearrange("(po pi) t -> pi po t", pi=P)   # [128, 6, T]
        wqkv_full = wqkv_d[:].rearrange("(po pi) c -> pi po c", pi=P)

        wqkv_sb = consts.tile([P, KT, 3 * D], BF16)
        xT_sb = {}
        for b in range(BPC):
            xT_sb[b] = xt_pool.tile([P, KT, SEQ], BF16, tag="xt", name=f"xT{b}")

        # qt0-half of batch-0 x and the first 4 qk weight slots go first so
        # the first attention unit's inputs land ~10us earlier
        for kt in range(KT):
            nc.sync.dma_start(out=wqkv_sb[:, kt, 0:512],
                              in_=wqkv_full[:, kt, 0:512])
            nc.sync.dma_start(out=xT_sb[0][:, kt, 0:NQ],
                              in_=xT_full[:, kt, 0:NQ])
        for kt in range(KT):
            nc.sync.dma_start(out=xT_sb[0][:, kt, NQ:SEQ],
                              in_=xT_full[:, kt, NQ:SEQ])
        for kt in range(KT):
            nc.sync.dma_start(out=wqkv_sb[:, kt, 2 * D:],
                              in_=wqkv_full[:, kt, 2 * D:])
        for kt in range(KT):
            nc.sync.dma_start(out=wqkv_sb[:, kt, 512:2 * D],
                              in_=wqkv_full[:, kt, 512:2 * D])
        for kt in range(KT):
            nc.sync.dma_start(out=xT_sb[1][:, kt, :],
                              in_=xT_full[:, kt, SEQ:2 * SEQ])
        wproj_sb = consts.tile([P, KT, D], BF16)
        nc.sync.dma_start(
            out=wproj_sb, in_=wproj_d[:].rearrange("(po pi) c -> pi po c", pi=P)
        )
        bias_sb = consts.tile([P, D], F32)
        b_ap = bias_d[:]
        bias_bcast = bass.AP(
            tensor=b_ap.tensor, offset=b_ap.offset, ap=[[0, P], *b_ap.ap]
        )
        nc.sync.dma_start(out=bias_sb, in_=bias_bcast)

        # ---------------- per-batch state ----------------
        qkT_sb, v_sb, outT_sb = {}, {}, {}
        for b in range(BPC):
            qkT_sb[b] = qk_pool.tile([P, 2 * KT, SEQ], BF16, tag="qkT",
                                     name=f"qkT{b}")
            v_sb[b] = v_pool.tile([P, KTT, H, 2 * HD], BF16, tag="v",
                                  name=f"v{b}")
            nc.gpsimd.memset(v_sb[b][:, :, :, HD:2 * HD], 1.0)
            outT_sb[b] = ot_pool.tile([P, KT, SEQ], BF16, tag="ot",
                                      name=f"ot{b}")

        # ---------------- filler units ----------------
        def qk_unit(b, slot, qt):
            ps = mm_ps.tile([P, NQ], F32, tag="mm", name=f"qk{b}_{slot}_{qt}")
            for kt in range(KT):
                nc.tensor.matmul(
                    ps,
                    lhsT=wqkv_sb[:, kt, slot * P:(slot + 1) * P],
                    rhs=xT_sb[b][:, kt, qt * NQ:(qt + 1) * NQ],
                    start=(kt == 0),
                    stop=(kt == KT - 1),
                    skip_group_check=True,
                )
            nc.vector.tensor_copy(
                out=qkT_sb[b][:, slot, qt * NQ:(qt + 1) * NQ], in_=ps
            )

        def v_unit(b, tt, nt):
            ps = mm_ps.tile([P, NQ], F32, tag="mm", name=f"v{b}_{tt}_{nt}")
            for kt in range(KT):
                nc.tensor.matmul(
                    ps[:, :NV],
                    lhsT=xT_sb[b][:, kt, tt * P:(tt + 1) * P],
                    rhs=wqkv_sb[:, kt, 2 * D + nt * NV:2 * D + (nt + 1) * NV],
                    start=(kt == 0),
                    stop=(kt == KT - 1),
                    skip_group_check=True,
                )
            nc.vector.tensor_copy(
                out=v_sb[b][:, tt, nt * 6:(nt + 1) * 6, 0:HD],
                in_=ps[:, :NV].rearrange("p (h d) -> p h d", d=HD),
            )

        def proj_unit(b, tt):
            y_sb = y_pool.tile([P, D], BF16, tag="y", name=f"y{b}_{tt}")
            for n0, nw in ((0, NQ), (NQ, D - NQ)):
                ps = mm_ps.tile([P, NQ], F32, tag="mm", name=f"p{b}_{tt}_{n0}")
                for dt2 in range(KT):
                    nc.tensor.matmul(
                        ps[:, :nw],
                        lhsT=outT_sb[b][:, dt2, tt * P:(tt + 1) * P],
                        rhs=wproj_sb[:, dt2, n0:n0 + nw],
                        start=(dt2 == 0),
                        stop=(dt2 == KT - 1),
                        skip_group_check=True,
                    )
                nc.vector.tensor_add(
                    out=y_sb[:, n0:n0 + nw],
                    in0=ps[:, :nw],
                    in1=bias_sb[:, n0:n0 + nw],
                )
            nc.sync.dma_start(
                out=y_d[b * SEQ + tt * P:b * SEQ + (tt + 1) * P, :], in_=y_sb
            )

        # need-order for a batch's QKV/V units; the qt=1 Q units (2h+1, 1)
        # are interleaved mid-list so the attention qt1 phase never stalls
        FILL_ORDER = [
            ("qk", 0, 0), ("qk", 1, 0), ("qk", 0, 1),
            ("v", 0, 0), ("v", 1, 0), ("v", 2, 0),
            ("qk", 2, 0), ("qk", 2, 1), ("qk", 3, 0),
            ("v", 3, 0), ("v", 4, 0),
            ("qk", 4, 0), ("qk", 4, 1), ("qk", 5, 0),
            ("v", 5, 0), ("v", 6, 0), ("v", 7, 0),
            ("qk", 6, 0), ("qk", 6, 1), ("qk", 7, 0),
            ("qk", 1, 1),
            ("v", 0, 1), ("v", 1, 1), ("v", 2, 1),
            ("qk", 8, 0), ("qk", 8, 1), ("qk", 9, 0),
            ("qk", 3, 1),
            ("v", 3, 1), ("v", 4, 1),
            ("qk", 10, 0), ("qk", 10, 1), ("qk", 11, 0),
            ("qk", 5, 1),
            ("v", 5, 1), ("v", 6, 1), ("v", 7, 1),
            ("qk", 7, 1), ("qk", 9, 1), ("qk", 11, 1),
        ]

        band(100000)
        for kind, a, c in FILL_ORDER:
            (qk_unit if kind == "qk" else v_unit)(0, a, c)
        band(200000)
        for kind, a, c in FILL_ORDER:
            (qk_unit if kind == "qk" else v_unit)(1, a, c)

        # ---------------- attention spine (lowest priorities) -------------
        def attn_unit(b, qt, hp):
            nt = hp // 3
            avs = [
                av_ps.tile([P, NQ], F32, tag="av", name=f"av{b}_{hp}_{qt}_{i}")
                for i in range(2)
            ]
            epairs = {}

            def st_exp(kt):
                stp = st_ps.tile(
                    [P, 2, NQ], F32, tag="st", name=f"st{b}_{hp}_{qt}_{kt}"
                )
                for hi in range(2):
                    base = hi * HD
                    nc.tensor.matmul(
                        stp[:, hi, :],
                        lhsT=qkT_sb[b][
                            base:base + HD, 2 * hp, kt * P:(kt + 1) * P
                        ],
                        rhs=qkT_sb[b][
                            base:base + HD, 2 * hp + 1, qt * NQ:(qt + 1) * NQ
                        ],
                        start=True,
                        stop=True,
                        skip_group_check=True,
                    )
                e_t = e_pool.tile(
                    [P, 2, NQ], BF16, tag="e", name=f"e{b}_{hp}_{qt}_{kt}"
                )
                nc.scalar.activation(
                    out=e_t,
                    in_=stp,
                    func=mybir.ActivationFunctionType.Exp,
                    scale=SCALE,
                )
                epairs[kt] = e_t

            def av(hi, kt):
                nc.tensor.matmul(
                    avs[hi],
                    lhsT=v_sb[b][:, kt, 2 * hp + hi, :],
                    rhs=epairs[kt][:, hi, :],
                    start=(kt == 0),
                    stop=(kt == KTT - 1),
                    skip_group_check=True,
                )

            def normalize(hi):
                base = hi * HD
                den = dn_pool.tile(
                    [HD, NQ], F32, tag="den", name=f"den{b}_{hp}_{qt}_{hi}"
                )
                nc.vector.tensor_copy(out=den, in_=avs[hi][HD:2 * HD, :])
                rb = rb_pool.tile(
                    [HD, NQ], F32, tag="rb", name=f"rb{b}_{hp}_{qt}_{hi}"
                )
                nc.vector.reciprocal_approx_fast(out=rb, in_=den)
                nc.vector.tensor_mul(
                    out=outT_sb[b][
                        base:base + HD, hp, qt * NQ:(qt + 1) * NQ
                    ],
                    in0=avs[hi][0:HD, :],
                    in1=rb,
                )

            st_exp(0)
            st_exp(1)
            for kt in range(2, KTT):
                st_exp(kt)
                av(0, kt - 2)
                av(1, kt - 2)
            for kt in (KTT - 2, KTT - 1):
                av(0, kt)
                av(1, kt)
            normalize(0)
            normalize(1)

        # Dependencies are tracked in EMISSION order (priorities only
        # reorder within the dep graph), so proj units — which read outT —
        # must be emitted after the spine units that write it.
        for b in range(BPC):
            for qt in range(QT):
                band(1000 + (2 * b + qt) * 1000)
                for hp in range(H // 2):
                    attn_unit(b, qt, hp)
                band(300000 + (2 * b + qt) * 10000)
                for tt in range(qt * 4, qt * 4 + 4):
                    proj_unit(b, tt)


def _build_program():
    nc = bacc.Bacc()
    xT_d = nc.declare_dram_parameter("xT", [D, T], BF16, isOutput=False)
    wqkv_d = nc.declare_dram_parameter("wqkv", [D, 3 * D], BF16, isOutput=False)
    wproj_d = nc.declare_dram_parameter("wproj", [D, D], BF16, isOutput=False)
    bias_d = nc.declare_dram_parameter("bias", [D], F32, isOutput=False)
    y_d = nc.declare_dram_parameter("y", [T, D], BF16, isOutput=True)

    with tile.TileContext(nc) as tc:
        _emit(tc, xT_d, wqkv_d, wproj_d, bias_d, y_d)
    nc.compile()
    return nc


_NC = None


def _get_nc():
    global _NC
    if _NC is None:
        _NC = _build_program()
    return _NC


def _qk_slot_perm():
    """Column permutation for the Q|K part of qkv_w: slot 2h <- K head-pair h,
    slot 2h+1 <- Q head-pair h."""
    perm = []
    for hp in range(H // 2):
        perm.extend(range(D + hp * P, D + (hp + 1) * P))      # K slot
        perm.extend(range(hp * P, (hp + 1) * P))              # Q slot
    return np.array(perm)


def _prep_in_maps(x, qkv_w, proj_w, proj_b):
    bf16 = ml_dtypes.bfloat16
    qkv_w = np.asarray(qkv_w)
    perm = _qk_slot_perm()
    wq_perm = np.concatenate([qkv_w[:, perm], qkv_w[:, 2 * D:]], axis=1)
    wq = np.ascontiguousarray(wq_perm.astype(bf16))
    wp = np.ascontiguousarray(np.asarray(proj_w).astype(bf16))
    pb = np.ascontiguousarray(np.asarray(proj_b).astype(np.float32))
    x = np.asarray(x)
    in_maps = []
    for c in range(N_CORES):
        xc = x[c * BPC:(c + 1) * BPC].reshape(T, D).astype(bf16)
        xTc = np.ascontiguousarray(xc.T)  # [D, T] bf16
        in_maps.append({"xT": xTc, "wqkv": wq, "wproj": wp, "bias": pb})
    return in_maps


def _run(x, qkv_w, proj_w, proj_b, **spmd_kwargs):
    nc = _get_nc()
    in_maps = _prep_in_maps(x, qkv_w, proj_w, proj_b)
    res = run_bass_kernel_spmd(nc, in_maps, core_ids=list(range(N_CORES)), **spmd_kwargs)
    y = np.stack([res.results[c]["y"] for c in range(N_CORES)])  # [8, T, D]
    return y.reshape(B, SEQ, D).astype(np.float32), res


def kernel(x, qkv_w, proj_w, proj_b):
    y, _ = _run(x, qkv_w, proj_w, proj_b)
    return y


# revision 24
# speedup vs baseline: 1.0080x; 1.0080x over previous
"""Trainium2 Bass kernel: batched multi-head self-attention (B=16, N=1024, D=768, H=12).

Strategy
--------
Data-parallel over the batch: 16 batches / 8 NeuronCores = 2 batches per core.
Each core runs an identical (SPMD) Bass program over its shard.

Per-core math, all matmuls in bf16 with fp32 PSUM accumulation:
  * Host pre-transposes x to xT [D, T] (T = 2048 local tokens) and casts
    x / qkv_w / proj_w to bf16.  qkv_w's Q|K columns are permuted on host
    into (K0,Q0,K1,Q1,...,K5,Q5) slot order so the first DMA chunks carry
    exactly what the first attention head-pair needs.
      - Q^T,K^T [c, tok] = matmul(lhsT=wqkv[:, slot], rhs=xT)
      - V [tok, c]       = matmul(lhsT=xT[:, tok-tile], rhs=wqkv_v)
      - S^T [k, q]       = matmul(lhsT=K^T_h [hd, k-tile], rhs=Q^T_h [hd, q])
        head pairs 2i/2i+1 live at partition bases 0/64 -> row-tiles
        (0,0)/(64,0) of the 64x128 PE config, streamed as adjacent pairs.
      - exp on ScalarE straight out of PSUM, bf16 into SBUF
      - out^T [hd, q]    = matmul(lhsT=[V_h | ones(64)], rhs=expT [k, q]);
        psum rows 64-127 = softmax denominator, replicated for free.
      - normalize: copy denominator to SBUF (reciprocal_approx_fast's
        BITWISE_NOT seed needs raw fp32 bits; PSUM reads don't deliver
        those on HW), reciprocal, tensor_mul -> outT (bf16)
      - y [tok, e]       = matmul(lhsT=out^T, rhs=proj_w) + bias -> bf16 DMA

Scheduling: the Tile framework scheduler is dependency-driven with a
priority heap per engine, so ordering is controlled via priority BANDS:
the attention spine (S^T pairs, exp, A@V, normalize) gets the lowest
priority numbers and preempts whenever its dependencies are satisfied;
QKV / V / proj matmul units sit in higher bands and automatically
backfill every PE bubble (exp latency, psum-ring waits, DMA waits).
Attention starts ~7us in: the first head-pair's K/Q/V units and their
DMA chunks are emitted first.

kernel() takes full unsharded inputs, shards on host, runs all 8 cores via
run_bass_kernel_spmd, and re-assembles the full output.
"""

import numpy as np
import ml_dtypes

import concourse.bass as bass
import concourse.mybir as mybir
import concourse.tile as tile
from concourse import bacc
from concourse.bass_utils import run_bass_kernel_spmd

BF16 = mybir.dt.bfloat16
F32 = mybir.dt.float32

N_CORES = 8
B, SEQ, D = 16, 1024, 768
H, HD = 12, 64
BPC = B // N_CORES            # batches per core
T = BPC * SEQ                 # tokens per core
P = 128
KT = D // P                   # 6 contraction sub-tiles of 128
NQ = 512                      # moving free-dim per matmul (1 psum bank of fp32)
QT = SEQ // NQ                # 2 query tiles per batch
KTT = SEQ // P                # 8 key-token tiles per batch
NV = 384                      # V-projection output tile (2 per 768)
SCALE = HD ** -0.5


def _emit(tc, xT_d, wqkv_d, wproj_d, bias_d, y_d):
    nc = tc.nc
    from contextlib import ExitStack

    def band(n):
        tc.cur_priority = n

    with ExitStack() as ctx:
        consts = ctx.enter_context(tc.tile_pool(name="consts", bufs=1))
        xt_pool = ctx.enter_context(tc.tile_pool(name="xt", bufs=2))
        qk_pool = ctx.enter_context(tc.tile_pool(name="qkT", bufs=2))
        v_pool = ctx.enter_context(tc.tile_pool(name="v", bufs=2))
        ot_pool = ctx.enter_context(tc.tile_pool(name="ot", bufs=2))
        e_pool = ctx.enter_context(tc.tile_pool(name="e", bufs=5))
        dn_pool = ctx.enter_context(tc.tile_pool(name="dn", bufs=2))
        rb_pool = ctx.enter_context(tc.tile_pool(name="rb", bufs=2))
        y_pool = ctx.enter_context(tc.tile_pool(name="y", bufs=2))
        mm_ps = ctx.enter_context(tc.tile_pool(name="mmps", bufs=2, space="PSUM"))
        st_ps = ctx.enter_context(tc.tile_pool(name="stps", bufs=2, space="PSUM"))
        av_ps = ctx.enter_context(tc.tile_pool(name="avps", bufs=2, space="PSUM"))

        # ---------------- DMA (need-ordered chunks), band 0 ----------------
        band(0)
        xT_full = xT_d[:].rearrange("(po pi) t -> pi po t", pi=P)   # [128, 6, T]
        wqkv_full = wqkv_d[:].rearrange("(po pi) c -> pi po c", pi=P)

        wqkv_sb = consts.tile([P, KT, 3 * D], BF16)
        xT_sb = {}
        for b in range(BPC):
            xT_sb[b] = xt_pool.tile([P, KT, SEQ], BF16, tag="xt", name=f"xT{b}")

        for kt in range(KT):
            nc.sync.dma_start(out=wqkv_sb[:, kt, 0:512],
                              in_=wqkv_full[:, kt, 0:512])
            nc.sync.dma_start(out=xT_sb[0][:, kt, :],
                              in_=xT_full[:, kt, 0:SEQ])
            nc.sync.dma_start(out=wqkv_sb[:, kt, 2 * D:],
                              in_=wqkv_full[:, kt, 2 * D:])
        for kt in range(KT):
            nc.sync.dma_start(out=wqkv_sb[:, kt, 512:2 * D],
                              in_=wqkv_full[:, kt, 512:2 * D])
        for kt in range(KT):
            nc.sync.dma_start(out=xT_sb[1][:, kt, :],
                              in_=xT_full[:, kt, SEQ:2 * SEQ])
        wproj_sb = consts.tile([P, KT, D], BF16)
        nc.sync.dma_start(
            out=wproj_sb, in_=wproj_d[:].rearrange("(po pi) c -> pi po c", pi=P)
        )
        bias_sb = consts.tile([P, D], F32)
        b_ap = bias_d[:]
        bias_bcast = bass.AP(
            tensor=b_ap.tensor, offset=b_ap.offset, ap=[[0, P], *b_ap.ap]
        )
        nc.sync.dma_start(out=bias_sb, in_=bias_bcast)

        # ---------------- per-batch state ----------------
        qkT_sb, v_sb, outT_sb = {}, {}, {}
        for b in range(BPC):
            qkT_sb[b] = qk_pool.tile([P, 2 * KT, SEQ], BF16, tag="qkT",
                                     name=f"qkT{b}")
            v_sb[b] = v_pool.tile([P, KTT, H, 2 * HD], BF16, tag="v",
                                  name=f"v{b}")
            nc.gpsimd.memset(v_sb[b][:, :, :, HD:2 * HD], 1.0)
            outT_sb[b] = ot_pool.tile([P, KT, SEQ], BF16, tag="ot",
                                      name=f"ot{b}")

        # ---------------- filler units ----------------
        def qk_unit(b, slot, qt):
            ps = mm_ps.tile([P, NQ], F32, tag="mm", name=f"qk{b}_{slot}_{qt}")
            for kt in range(KT):
                nc.tensor.matmul(
                    ps,
                    lhsT=wqkv_sb[:, kt, slot * P:(slot + 1) * P],
                    rhs=xT_sb[b][:, kt, qt * NQ:(qt + 1) * NQ],
                    start=(kt == 0),
                    stop=(kt == KT - 1),
                    skip_group_check=True,
                )
            nc.vector.tensor_copy(
                out=qkT_sb[b][:, slot, qt * NQ:(qt + 1) * NQ], in_=ps
            )

        def v_unit(b, tt, nt):
            ps = mm_ps.tile([P, NQ], F32, tag="mm", name=f"v{b}_{tt}_{nt}")
            for kt in range(KT):
                nc.tensor.matmul(
                    ps[:, :NV],
                    lhsT=xT_sb[b][:, kt, tt * P:(tt + 1) * P],
                    rhs=wqkv_sb[:, kt, 2 * D + nt * NV:2 * D + (nt + 1) * NV],
                    start=(kt == 0),
                    stop=(kt == KT - 1),
                    skip_group_check=True,
                )
            nc.vector.tensor_copy(
                out=v_sb[b][:, tt, nt * 6:(nt + 1) * 6, 0:HD],
                in_=ps[:, :NV].rearrange("p (h d) -> p h d", d=HD),
            )

        def proj_unit(b, tt):
            y_sb = y_pool.tile([P, D], BF16, tag="y", name=f"y{b}_{tt}")
            for n0, nw in ((0, NQ), (NQ, D - NQ)):
                ps = mm_ps.tile([P, NQ], F32, tag="mm", name=f"p{b}_{tt}_{n0}")
                for dt2 in range(KT):
                    nc.tensor.matmul(
                        ps[:, :nw],
                        lhsT=outT_sb[b][:, dt2, tt * P:(tt + 1) * P],
                        rhs=wproj_sb[:, dt2, n0:n0 + nw],
                        start=(dt2 == 0),
                        stop=(dt2 == KT - 1),
                        skip_group_check=True,
                    )
                nc.vector.tensor_add(
                    out=y_sb[:, n0:n0 + nw],
                    in0=ps[:, :nw],
                    in1=bias_sb[:, n0:n0 + nw],
                )
            nc.sync.dma_start(
                out=y_d[b * SEQ + tt * P:b * SEQ + (tt + 1) * P, :], in_=y_sb
            )

        # need-order for a batch's QKV/V units; the qt=1 Q units (2h+1, 1)
        # are interleaved mid-list so the attention qt1 phase never stalls
        FILL_ORDER = [
            ("qk", 0, 0), ("qk", 0, 1), ("qk", 1, 0),
            ("v", 0, 0), ("v", 1, 0), ("v", 2, 0),
            ("qk", 2, 0), ("qk", 2, 1), ("qk", 3, 0),
            ("v", 3, 0), ("v", 4, 0),
            ("qk", 4, 0), ("qk", 4, 1), ("qk", 5, 0),
            ("v", 5, 0), ("v", 6, 0), ("v", 7, 0),
            ("qk", 6, 0), ("qk", 6, 1), ("qk", 7, 0),
            ("qk", 1, 1),
            ("v", 0, 1), ("v", 1, 1), ("v", 2, 1),
            ("qk", 8, 0), ("qk", 8, 1), ("qk", 9, 0),
            ("qk", 3, 1),
            ("v", 3, 1), ("v", 4, 1),
            ("qk", 10, 0), ("qk", 10, 1), ("qk", 11, 0),
            ("qk", 5, 1),
            ("v", 5, 1), ("v", 6, 1), ("v", 7, 1),
            ("qk", 7, 1), ("qk", 9, 1), ("qk", 11, 1),
        ]

        band(100000)
        for kind, a, c in FILL_ORDER:
            (qk_unit if kind == "qk" else v_unit)(0, a, c)
        band(200000)
        for kind, a, c in FILL_ORDER:
            (qk_unit if kind == "qk" else v_unit)(1, a, c)

        # ---------------- attention spine (lowest priorities) -------------
        def attn_unit(b, qt, hp):
            nt = hp // 3
            avs = [
                av_ps.tile([P, NQ], F32, tag="av", name=f"av{b}_{hp}_{qt}_{i}")
                for i in range(2)
            ]
            epairs = {}

            def st_exp(kt):
                stp = st_ps.tile(
                    [P, 2, NQ], F32, tag="st", name=f"st{b}_{hp}_{qt}_{kt}"
                )
                for hi in range(2):
                    base = hi * HD
                    nc.tensor.matmul(
                        stp[:, hi, :],
                        lhsT=qkT_sb[b][
                            base:base + HD, 2 * hp, kt * P:(kt + 1) * P
                        ],
                        rhs=qkT_sb[b][
                            base:base + HD, 2 * hp + 1, qt * NQ:(qt + 1) * NQ
                        ],
                        start=True,
                        stop=True,
                        skip_group_check=True,
                    )
                e_t = e_pool.tile(
                    [P, 2, NQ], BF16, tag="e", name=f"e{b}_{hp}_{qt}_{kt}"
                )
                nc.scalar.activation(
                    out=e_t,
                    in_=stp,
                    func=mybir.ActivationFunctionType.Exp,
                    scale=SCALE,
                )
                epairs[kt] = e_t

            def av(hi, kt):
                nc.tensor.matmul(
                    avs[hi],
                    lhsT=v_sb[b][:, kt, 2 * hp + hi, :],
                    rhs=epairs[kt][:, hi, :],
                    start=(kt == 0),
                    stop=(kt == KTT - 1),
                    skip_group_check=True,
                )

            def normalize(hi):
                base = hi * HD
                den = dn_pool.tile(
                    [HD, NQ], F32, tag="den", name=f"den{b}_{hp}_{qt}_{hi}"
                )
                nc.vector.tensor_copy(out=den, in_=avs[hi][HD:2 * HD, :])
                rb = rb_pool.tile(
                    [HD, NQ], F32, tag="rb", name=f"rb{b}_{hp}_{qt}_{hi}"
                )
                nc.vector.reciprocal_approx_fast(out=rb, in_=den)
                nc.vector.tensor_mul(
                    out=outT_sb[b][
                        base:base + HD, hp, qt * NQ:(qt + 1) * NQ
                    ],
                    in0=avs[hi][0:HD, :],
                    in1=rb,
                )

            st_exp(0)
            st_exp(1)
            for kt in range(2, KTT):
                st_exp(kt)
                av(0, kt - 2)
                av(1, kt - 2)
            for kt in (KTT - 2, KTT - 1):
                av(0, kt)
                av(1, kt)
            normalize(0)
            normalize(1)

        # Dependencies are tracked in EMISSION order (priorities only
        # reorder within the dep graph), so proj units — which read outT —
        # must be emitted after the spine units that write it.
        for b in range(BPC):
            for qt in range(QT):
                band(1000 + (2 * b + qt) * 1000)
                for hp in range(H // 2):
                    attn_unit(b, qt, hp)
                band(300000 + (2 * b + qt) * 10000)
                for tt in range(qt * 4, qt * 4 + 4):
                    proj_unit(b, tt)


def _build_program():
    nc = bacc.Bacc()
    xT_d = nc.declare_dram_parameter("xT", [D, T], BF16, isOutput=False)
    wqkv_d = nc.declare_dram_parameter("wqkv", [D, 3 * D], BF16, isOutput=False)
    wproj_d = nc.declare_dram_parameter("wproj", [D, D], BF16, isOutput=False)
    bias_d = nc.declare_dram_parameter("bias", [D], F32, isOutput=False)
    y_d = nc.declare_dram_parameter("y", [T, D], BF16, isOutput=True)

    with tile.TileContext(nc) as tc:
        _emit(tc, xT_d, wqkv_d, wproj_d, bias_d, y_d)
    nc.compile()
    return nc


_NC = None


def _get_nc():
    global _NC
    if _NC is None:
        _NC = _build_program()
    return _NC


def _qk_slot_perm():
    """Column permutation for the Q|K part of qkv_w: slot 2h <- K head-pair h,
    slot 2h+1 <- Q head-pair h."""
    perm = []
    for hp in range(H // 2):
        perm.extend(range(D + hp * P, D + (hp + 1) * P))      # K slot
        perm.extend(range(hp * P, (hp + 1) * P))              # Q slot
    return np.array(perm)


def _prep_in_maps(x, qkv_w, proj_w, proj_b):
    bf16 = ml_dtypes.bfloat16
    qkv_w = np.asarray(qkv_w)
    perm = _qk_slot_perm()
    wq_perm = np.concatenate([qkv_w[:, perm], qkv_w[:, 2 * D:]], axis=1)
    wq = np.ascontiguousarray(wq_perm.astype(bf16))
    wp = np.ascontiguousarray(np.asarray(proj_w).astype(bf16))
    pb = np.ascontiguousarray(np.asarray(proj_b).astype(np.float32))
    x = np.asarray(x)
    in_maps = []
    for c in range(N_CORES):
        xc = x[c * BPC:(c + 1) * BPC].reshape(T, D).astype(bf16)
        xTc = np.ascontiguousarray(xc.T)  # [D, T] bf16
        in_maps.append({"xT": xTc, "wqkv": wq, "wproj": wp, "bias": pb})
    return in_maps


def _run(x, qkv_w, proj_w, proj_b, **spmd_kwargs):
    nc = _get_nc()
    in_maps = _prep_in_maps(x, qkv_w, proj_w, proj_b)
    res = run_bass_kernel_spmd(nc, in_maps, core_ids=list(range(N_CORES)), **spmd_kwargs)
    y = np.stack([res.results[c]["y"] for c in range(N_CORES)])  # [8, T, D]
    return y.reshape(B, SEQ, D).astype(np.float32), res


def kernel(x, qkv_w, proj_w, proj_b):
    y, _ = _run(x, qkv_w, proj_w, proj_b)
    return y
